# revision 11
# baseline (speedup 1.0000x reference)
"""AdaptiveRankingLoss distributed Bass kernel for 8 TRN2 NeuronCores.

Math
----
reference loss = sum_{i<j, t_i != t_j} w_ij * relu(margin_ij - sign(t_i - t_j)*(p_i - p_j))
                 / count,
  margin = 0.1 * clip(|t_i - t_j|, 0.1, 1.0),  w = 1/(1 + u_i + u_j).

The summand is symmetric under i<->j, and splitting by the sign of
a = t_j - t_i gives an exactly equivalent full-matrix form with no sign(),
no abs() and no triangular mask:

    numerator = sum_{all i,j} [a_ij > 0] * w_ij * relu(clip(0.1*a_ij, .01, .1) - (p_j - p_i))

Ties (a == 0, including the diagonal) contribute exactly 0 via the
indicator, and `count` is computed exactly on the host from duplicate
analysis of t.

Device mapping (per core: 1024 rows x 8192 cols of the pair matrix)
------------------------------------------------------------------
* one custom 8-stage DVE op produces v = [a>0]*relu(clip(0.1a,.01,.1)-b)
  per element (fp32 internal, bf16 out), streaming the broadcast column
  vectors with the row values as per-partition scalars.
* the weight w = 1/(1+u_i+u_j) is applied through a degree-6 bilinear
  polynomial 1/(2+z) ~ p(z), z = x_i + x_j, x = u - 0.5:
      w_ij ~ sum_n Phi_n(x_i) * x_j^n
  so  sum_ij v_ij w_ij = sum_{n,j} X[n,j] * Psi[n,j]  with
      X[n,j] = sum_i Phi_n(x_i) v_ij   (TensorEngine matmul, PSUM accum)
      Psi[n,j] = x_j^n.
* a fused tensor_tensor_reduce drains each PSUM chunk into a running
  [7,1] accumulator; the host sums 7 values per core and divides by count.
"""

import numpy as np

import concourse.bass as bass
import concourse.bacc as bacc
import concourse.mybir as mybir
import concourse.tile as tile
from concourse.bass_utils import run_bass_kernel_spmd
from concourse import dve_ops
from concourse.dve_spec import (
    Spec,
    Src0,
    Src1,
    C0,
    C1,
    C2,
    Zero,
    relu,
    maxx,
    minn,
    lower,
    _has_src1,
)
from concourse.dve_uop import DveOpSpec

F32 = mybir.dt.float32
BF16 = mybir.dt.bfloat16

N = 8192          # problem size (hardcoded per spec)
NCORES = 8
P = 128           # SBUF partitions
R = N // NCORES   # rows per core (1024)
RT = R // P       # row tiles per core (8)
FC = 1024         # column chunk
NCH = N // FC     # chunks (8)
DEG = 6           # weight polynomial degree
K = DEG + 1
MMF = 512         # matmul free-dim tile

# Inputs are sorted by target on the host and rows are strided across cores
# (core c gets sorted rows c, c+8, ...). Row-tile r of any core then covers
# sorted positions >= 1024*r, so column chunks c < r satisfy t_j <= t_i
# everywhere -> the [a>0] indicator is identically 0 and the chunk is skipped
# for that tile. Bit-exact with the unskipped computation.


# --------------------------------------------------------------------------
# custom DVE op: v = [Src0 - C0 > 0] * relu(clip(Src0 - C0, C2^2, C2) - (Src1 - C1))
# Src0 = 0.1*t_col, C0 = 0.1*t_row, Src1 = p_col, C1 = p_row, C2 = 0.1.
# --------------------------------------------------------------------------
_ARL_NAME = "ARL_MAIN_V1"


def _arl_reference(in0, in1, s0, s1, imm2):
    a = in0 - s0
    m = np.clip(a, np.float32(imm2) * np.float32(imm2), imm2)
    return (a > 0).astype(np.float32) * np.maximum(m - (in1 - s1), 0.0)


def _register_arl_op():
    for op in dve_ops.OPS:
        if op.name == _ARL_NAME:
            return op
    a = Src0 - C0
    m = minn(maxx(a, C2 * C2), C2)
    h = relu(m - (Src1 - C1))
    spec = Spec(body=(a > Zero) * h, reference=_arl_reference)
    row = dve_ops._CUSTOM_DVE_ROW_BASE + len(dve_ops.OPS)
    assert row < 0x20, "custom-DVE row overflow"
    dve_ops._SUB_OPCODE_FOR_NAME[_ARL_NAME] = row
    shas = {}
    for ver in ("v3", "v4"):
        try:
            uops = lower(spec, ver=ver)
            shas[ver] = DveOpSpec(
                name=_ARL_NAME, opcode=row, uops=uops, rd1_en=_has_src1(spec)
            ).sha(ver)
        except Exception:
            pass
    op = dve_ops.DveOp(_ARL_NAME, spec, subdim=False, uops_sha=shas)
    dve_ops.OPS.append(op)
    dve_ops.CUSTOM_DVE_SPECS[_ARL_NAME] = spec
    return op


ARL_MAIN = _register_arl_op()


# --------------------------------------------------------------------------
# degree-6 bilinear split of w = 1/(1+u_i+u_j) = 1/(2 + x_i + x_j), x = u-.5
# --------------------------------------------------------------------------
def _acoef_matrix() -> np.ndarray:
    from numpy.polynomial import chebyshev as _C
    from math import comb

    nodes = np.cos((2 * np.arange(DEG + 1) + 1) / (2 * (DEG + 1)) * np.pi)
    ch = _C.chebfit(nodes, 1.0 / (2.0 + nodes), DEG)
    c = _C.cheb2poly(ch)  # power-basis coeffs of p(z) ~ 1/(2+z) on [-1,1]
    A = np.zeros((K, K), np.float64)
    for mm in range(K):
        for nn in range(K):
            if mm + nn <= DEG:
                A[mm, nn] = c[mm + nn] * comb(mm + nn, mm)
    return A.astype(np.float32)


_ACOEF = _acoef_matrix()


# --------------------------------------------------------------------------
# device graph builder
# --------------------------------------------------------------------------
def _build_nc():
    from contextlib import ExitStack

    nc = bacc.Bacc(None, target_bir_lowering=False, debug=False)

    t01_ext = nc.declare_dram_parameter("t01col", [N], F32, isOutput=False)
    p_ext = nc.declare_dram_parameter("pcol", [N], F32, isOutput=False)
    u_ext = nc.declare_dram_parameter("ucol", [N], F32, isOutput=False)
    t01r_ext = nc.declare_dram_parameter("t01row", [R], F32, isOutput=False)
    pr_ext = nc.declare_dram_parameter("prow", [R], F32, isOutput=False)
    ur_ext = nc.declare_dram_parameter("urow", [R], F32, isOutput=False)
    a_ext = nc.declare_dram_parameter("acoef", [K, K], F32, isOutput=False)
    out_ext = nc.declare_dram_parameter("out", [K], F32, isOutput=True)

    with tile.TileContext(nc) as tc, ExitStack() as ctx:
        constp = ctx.enter_context(tc.tile_pool(name="const", bufs=1))
        colp = ctx.enter_context(tc.tile_pool(name="cols", bufs=1))
        vp = ctx.enter_context(tc.tile_pool(name="v", bufs=4))
        pp = ctx.enter_context(tc.tile_pool(name="psum", bufs=2, space="PSUM"))
        sp = ctx.enter_context(tc.tile_pool(name="small", bufs=1))
        dramp = ctx.enter_context(tc.tile_pool(name="dram", bufs=1, space="DRAM"))

        # ---- per-row scalars [P, RT] (gate MAIN: issue first) ----
        t01row_sb = constp.tile([P, RT], F32)
        nc.sync.dma_start(t01row_sb[:], t01r_ext[:].rearrange("(r p) -> p r", p=P))
        prow_sb = constp.tile([P, RT], F32)
        nc.sync.dma_start(prow_sb[:], pr_ext[:].rearrange("(r p) -> p r", p=P))

        # ---- column chunks, broadcast to all partitions (issue in
        # processing order: biggest/last chunk first) ----
        t01cs, pccs = {}, {}
        for c in reversed(range(NCH)):
            t01cs[c] = colp.tile([P, FC], F32, tag=f"tcol{c}", name=f"t01c{c}")
            nc.sync.dma_start(
                t01cs[c][:],
                bass.AP(tensor=t01_ext, offset=c * FC, ap=[[0, P], [1, FC]]),
            )
            pccs[c] = colp.tile([P, FC], F32, tag=f"pcol{c}", name=f"pcc{c}")
            nc.sync.dma_start(
                pccs[c][:],
                bass.AP(tensor=p_ext, offset=c * FC, ap=[[0, P], [1, FC]]),
            )

        # ---- remaining prep (gates matmul / TTR only) ----
        urow_sb = constp.tile([P, RT], F32)
        nc.sync.dma_start(urow_sb[:], ur_ext[:].rearrange("(r p) -> p r", p=P))
        abuf = constp.tile([P, K, K], F32)
        a_src = bass.AP(tensor=a_ext, offset=0, ap=[[0, P], [K, K], [1, K]])
        nc.sync.dma_start(abuf[:], a_src)

        # Phi[p, r, n] = sum_m A[m, n] * x_row^m  (Horner), bf16
        xrow = sp.tile([P, RT], F32)
        nc.vector.tensor_scalar_sub(xrow[:], urow_sb[:], 0.5)
        phit = sp.tile([P, RT, K], F32)
        nc.vector.tensor_copy(
            phit[:], abuf[:, DEG : DEG + 1, :].broadcast_to([P, RT, K])
        )
        xrow_b = xrow[:, :, None].broadcast_to([P, RT, K])
        for m in range(DEG - 1, -1, -1):
            nc.vector.tensor_mul(phit[:], phit[:], xrow_b)
            nc.vector.tensor_add(
                phit[:], phit[:], abuf[:, m : m + 1, :].broadcast_to([P, RT, K])
            )
        phib = constp.tile([P, RT, K], BF16)
        nc.vector.tensor_copy(phib[:], phit[:])

        # Psi[n, j] = x_j^n, built in [P, 64] layout, bounced via DRAM
        FB = N // P  # 64
        psi_dram = dramp.tile([K, N], F32)
        u64 = sp.tile([P, FB], F32)
        nc.sync.dma_start(u64[:], u_ext[:].rearrange("(p f) -> p f", p=P))
        x64 = sp.tile([P, FB], F32)
        nc.vector.tensor_scalar_sub(x64[:], u64[:], 0.5)
        ones64 = sp.tile([P, FB], F32)
        nc.vector.memset(ones64[:], 1.0)
        nc.sync.dma_start(psi_dram[0, :].rearrange("(p f) -> p f", p=P), ones64[:])
        nc.sync.dma_start(psi_dram[1, :].rearrange("(p f) -> p f", p=P), x64[:])
        prev = x64
        for n in range(2, K):
            nxt = sp.tile([P, FB], F32, tag=f"pw{n}")
            nc.vector.tensor_mul(nxt[:], prev[:], x64[:])
            nc.sync.dma_start(psi_dram[n, :].rearrange("(p f) -> p f", p=P), nxt[:])
            prev = nxt
        psi_sb = constp.tile([K, N], F32)
        nc.sync.dma_start(psi_sb[:], psi_dram[:, :])

        # ---- running accumulator ----
        acc = sp.tile([K, 1], F32)
        ttr_scr = sp.tile([K, FC], F32)

        # ---- main pairwise loop (largest chunk first) ----
        for idx, c in enumerate(reversed(range(NCH))):
            t01c, pcc = t01cs[c], pccs[c]
            Xc = pp.tile([K, FC], F32)
            ntiles = min(RT, c + 1)  # triangular skip: tile r active iff r <= c
            for r in range(ntiles):
                v = vp.tile([P, FC], BF16, tag="v")
                nc.vector._custom_dve(
                    ARL_MAIN,
                    out=v[:],
                    in0=t01c[:],
                    in1=pcc[:],
                    s0=t01row_sb[:, r : r + 1],
                    s1=prow_sb[:, r : r + 1],
                    imm2=0.1,
                )
                for kk in range(FC // MMF):
                    nc.tensor.matmul(
                        Xc[:, kk * MMF : (kk + 1) * MMF],
                        phib[:, r, :],
                        v[:, kk * MMF : (kk + 1) * MMF],
                        start=(r == 0),
                        stop=(r == ntiles - 1),
                    )
            nc.vector._custom_dve(
                dve_ops.TENSOR_TENSOR_REDUCE,
                out=ttr_scr[:],
                in0=Xc[:],
                in1=psi_sb[:, c * FC : (c + 1) * FC],
                s0=(0.0 if idx == 0 else acc[:]),
                s1=1.0,
                accum_out=acc[:],
            )

        nc.sync.dma_start(out_ext[:], acc[:, 0:1])

    nc.compile()
    return nc


_NC_CACHE = None


def _get_nc():
    global _NC_CACHE
    if _NC_CACHE is None:
        _NC_CACHE = _build_nc()
    return _NC_CACHE


def _exact_count(t: np.ndarray) -> int:
    n = t.shape[0]
    _, cnts = np.unique(t, return_counts=True)
    dup = int(sum(int(c) * (int(c) - 1) // 2 for c in cnts[cnts > 1]))
    return n * (n - 1) // 2 - dup


def _make_in_maps(predictions, targets, uncertainties):
    t = np.ascontiguousarray(np.asarray(targets, np.float32))
    p = np.ascontiguousarray(np.asarray(predictions, np.float32))
    u = np.ascontiguousarray(np.asarray(uncertainties, np.float32))
    # sort by target (loss is permutation invariant); stride rows across
    # cores so every core sees the same triangular-skip schedule.
    order = np.argsort(t, kind="stable")
    ts, ps, us = t[order], p[order], u[order]
    t01 = (np.float32(0.1) * ts).astype(np.float32)
    in_maps = []
    for i in range(NCORES):
        in_maps.append(
            {
                "t01col": t01,
                "pcol": ps,
                "ucol": us,
                "t01row": np.ascontiguousarray(t01[i::NCORES]),
                "prow": np.ascontiguousarray(ps[i::NCORES]),
                "urow": np.ascontiguousarray(us[i::NCORES]),
                "acoef": _ACOEF,
            }
        )
    return in_maps, t


def _run_device(in_maps, trace=False, **kw):
    nc = _get_nc()
    return run_bass_kernel_spmd(
        nc, in_maps, core_ids=list(range(NCORES)), trace=trace, **kw
    )


def kernel(predictions, targets, uncertainties):
    in_maps, t = _make_in_maps(predictions, targets, uncertainties)
    res = _run_device(in_maps)
    total = np.float64(0.0)
    for r in res.results:
        total += np.asarray(r["out"], np.float64).sum()
    count = _exact_count(t)
    return np.asarray(total / max(count, 1), dtype=np.float32)
